# revision 1
# baseline (speedup 1.0000x reference)
# Trainium2 Bass kernel for nn_AxialAttention (8 NeuronCores, head/W-parallel).
#
# Sharding: the W axis (axis=2, the vmapped axis) is split into 8 contiguous
# slices of 32 columns, one per core. Every part of the computation (the four
# 1x1-conv GEMMs, the per-(head, w) axial attention, the embedding terms) is
# independent across w, so there are no collectives; the small weight matrices
# and embedding tables are replicated to every core.
#
# Per-core math for one w column (all heads):
#   qsT[x, (h c)] = query[:, :, w].T @ (Wq.T / 16)    (scale folded into Wq)
#   khT[x, (h c)] = key_[:, :, w].T @ Wk.T
#   vh [(h c), x] = Wv @ value[:, :, w]
#   logits_h[C, c] = khT_h.T @ qsT_h + q_emb.T @ qsT_h + k_emb.T @ khT_h
#   E = exp(logits)             (max-subtraction unnecessary: |logits| < ~2)
#   U_h = E_h.T @ [vh_h + ve | 1]          (ones column gives the softmax
#   attn_h = U_h[:, :256] / U_h[:, 256]     denominator for free)
#   out[:, :, w] = Wo @ attn
#
# Heads are packed even/odd into the two 64-partition halves so the per-head
# 64x64 logits matmuls and the 64-row attention matmuls run as concurrent
# PE row/column tiles (tile_position diagonal packing). All matmuls run in
# bf16 with fp32 PSUM accumulation (measured 3.3e-3 absmax-relative error);
# large PSUM->SBUF evacuations are split across the Scalar and Vector engines
# to halve PSUM-slot release latency.

import numpy as np

H = 8          # heads
QK = 64        # per-head qk/vo channels
C = 512        # io channels
X = 256        # spatial H (attention contraction axis)
W = 256        # spatial W (vmapped axis, sharded)
N_CORES = 8
WC = W // N_CORES   # w columns per core
PAIRS = WC // 2

_CACHE = {}


def _build_program():
    import concourse.mybir as mybir
    import concourse.tile as tile
    from concourse import bacc

    f32 = mybir.dt.float32
    bf16 = mybir.dt.bfloat16
    AF = mybir.ActivationFunctionType

    nc = bacc.Bacc("TRN2", target_bir_lowering=False, debug=False,
                   num_devices=N_CORES)

    qin = nc.dram_tensor("qin", [PAIRS, C, 2, X], bf16, kind="ExternalInput").ap()
    kin = nc.dram_tensor("kin", [PAIRS, C, 2, X], bf16, kind="ExternalInput").ap()
    vin = nc.dram_tensor("vin", [PAIRS, C, 2, X], bf16, kind="ExternalInput").ap()
    wqt = nc.dram_tensor("wqt", [C, C], bf16, kind="ExternalInput").ap()
    wkt = nc.dram_tensor("wkt", [C, C], bf16, kind="ExternalInput").ap()
    wvt = nc.dram_tensor("wvt", [C, C], bf16, kind="ExternalInput").ap()
    wot = nc.dram_tensor("wot", [C, C], bf16, kind="ExternalInput").ap()
    qe8 = nc.dram_tensor("qe8", [X, H * QK], bf16, kind="ExternalInput").ap()
    ke2 = nc.dram_tensor("ke2", [X, 2 * QK], bf16, kind="ExternalInput").ap()
    vet = nc.dram_tensor("vet", [QK, X], f32, kind="ExternalInput").ap()
    oned = nc.dram_tensor("oned", [128, 4], bf16, kind="ExternalInput").ap()
    out = nc.dram_tensor("out", [C, WC, X], f32, kind="ExternalOutput").ap()

    KT = C // 128   # 4 contraction tiles of the channel dim
    XT = X // 128   # 2 tiles of the spatial-x dim

    with tile.TileContext(nc) as tc:
        with (
            tc.tile_pool(name="consts", bufs=1) as consts,
            tc.tile_pool(name="inp", bufs=3) as inp,
            tc.tile_pool(name="qkt", bufs=2) as qkt,
            tc.tile_pool(name="mid", bufs=2) as mid,
            tc.tile_pool(name="small", bufs=8) as small,
            tc.tile_pool(name="psA", bufs=3, space="PSUM") as psA,
            tc.tile_pool(name="psVL", bufs=2, space="PSUM") as psVL,
            tc.tile_pool(name="psU", bufs=3, space="PSUM") as psU,
        ):
            def load_inputs(pair):
                q_t = inp.tile([128, KT, 2, X], bf16, tag="q_t")
                nc.sync.dma_start(
                    q_t[:], qin[pair].rearrange("(kt p) w x -> p kt (w x)", p=128))
                k_t = inp.tile([128, KT, 2, X], bf16, tag="k_t")
                nc.sync.dma_start(
                    k_t[:], kin[pair].rearrange("(kt p) w x -> p kt (w x)", p=128))
                v_t = inp.tile([128, KT, 2, X], bf16, tag="v_t")
                nc.sync.dma_start(
                    v_t[:], vin[pair].rearrange("(kt p) w x -> p kt (w x)", p=128))
                return q_t, k_t, v_t

            # pair-0 inputs first so the PE can start ASAP; q is split per
            # k-tile so the first matmul only waits for one chunk. Constants
            # go on the ACT HWDGE ring so the two DMA streams run in parallel.
            q0 = inp.tile([128, KT, 2, X], bf16, tag="q_t")
            for kt in range(KT):
                nc.sync.dma_start(
                    q0[:, kt, :, :],
                    qin[0, kt * 128:(kt + 1) * 128].rearrange("p w x -> p (w x)"))
            k0 = inp.tile([128, KT, 2, X], bf16, tag="k_t")
            nc.sync.dma_start(
                k0[:], kin[0].rearrange("(kt p) w x -> p kt (w x)", p=128))
            v0 = inp.tile([128, KT, 2, X], bf16, tag="v_t")
            nc.sync.dma_start(
                v0[:], vin[0].rearrange("(kt p) w x -> p kt (w x)", p=128))
            prefetched = (q0, k0, v0)

            wq_sb = consts.tile([128, KT, C], bf16)
            nc.scalar.dma_start(wq_sb[:], wqt.rearrange("(kt p) o -> p kt o", p=128))
            wk_sb = consts.tile([128, KT, C], bf16)
            nc.scalar.dma_start(wk_sb[:], wkt.rearrange("(kt p) o -> p kt o", p=128))
            wv_sb = consts.tile([128, KT, C], bf16)
            nc.scalar.dma_start(wv_sb[:], wvt.rearrange("(kt p) o -> p kt o", p=128))
            wo_sb = consts.tile([128, KT, C], bf16)
            nc.scalar.dma_start(wo_sb[:], wot.rearrange("(kt p) o -> p kt o", p=128))
            qe8_sb = consts.tile([128, XT, H * QK], bf16)
            nc.scalar.dma_start(qe8_sb[:], qe8.rearrange("(xt p) m -> p xt m", p=128))
            ke_sb = consts.tile([128, XT, 2 * QK], bf16)
            nc.scalar.dma_start(ke_sb[:], ke2.rearrange("(xt p) m -> p xt m", p=128))
            ve_sb = consts.tile([128, X], f32)
            nc.scalar.dma_start(ve_sb[0:QK, :], vet[:])
            nc.scalar.dma_start(ve_sb[QK:128, :], vet[:])
            ones_sb = consts.tile([128, 2, 2], bf16)
            nc.scalar.dma_start(ones_sb[:], oned.rearrange("p (a b) -> p a b", a=2))

            for pair in range(PAIRS):
                w0 = pair * 2
                q_t, k_t, v_t = prefetched if pair == 0 else load_inputs(pair)

                # --- q/k projections, transposed layout: qsT/khT [x, (h c)] ---
                qsT = qkt.tile([128, 2, XT, C], bf16)   # [x_p, w, xt, o]
                khT = qkt.tile([128, 2, XT, C], bf16)
                khq = qkt.tile([128, 2, XT, C], bf16)   # khT + q_emb (folds t2)
                for wi in range(2):
                    for xt in range(XT):
                        pq = psA.tile([128, C], f32, tag="mm")
                        for kt in range(KT):
                            nc.tensor.matmul(
                                pq[:],
                                q_t[:, kt, wi, xt * 128:(xt + 1) * 128],
                                wq_sb[:, kt, :],
                                start=(kt == 0), stop=(kt == KT - 1))
                        nc.scalar.activation(qsT[:, wi, xt, 0:256], pq[:, 0:256],
                                             AF.Copy)
                        nc.vector.tensor_copy(qsT[:, wi, xt, 256:512],
                                              pq[:, 256:512])
                        pk = psA.tile([128, C], f32, tag="mm")
                        for kt in range(KT):
                            nc.tensor.matmul(
                                pk[:],
                                k_t[:, kt, wi, xt * 128:(xt + 1) * 128],
                                wk_sb[:, kt, :],
                                start=(kt == 0), stop=(kt == KT - 1))
                        nc.vector.tensor_copy(khT[:, wi, xt, 0:256], pk[:, 0:256])
                        nc.scalar.activation(khT[:, wi, xt, 256:512],
                                             pk[:, 256:512], AF.Copy)
                        nc.gpsimd.tensor_add(khq[:, wi, xt, :],
                                             khT[:, wi, xt, :], qe8_sb[:, xt, :])

                # --- v projection + ve add + ones column ---
                vplus = mid.tile([128, KT, 2, X + 2], bf16)  # [c2_p, head-pair, w, x+2]
                for ot in range(KT):
                    pv = psVL.tile([128, 2, X], f32, tag="vl")
                    for kt in range(KT):
                        nc.tensor.matmul(
                            pv[:],
                            wv_sb[:, kt, ot * 128:(ot + 1) * 128],
                            v_t[:, kt, :, :],
                            start=(kt == 0), stop=(kt == KT - 1))
                    for wi in range(2):
                        nc.vector.tensor_add(
                            vplus[:, ot, wi, 0:X], pv[:, wi, :], ve_sb[:])
                    nc.vector.tensor_copy(vplus[:, ot, :, X:X + 2], ones_sb[:])

                # --- per-w attention ---
                attn = mid.tile([128, KT, 2, X], bf16)  # [(h c)_p, kt, w, x]
                for wi in range(2):
                    pl = psVL.tile([128, C], f32, tag="vl")
                    # k_emb term, all heads at once (dup'd table)
                    nc.tensor.matmul(pl[:], ke_sb[:, 0, :], khT[:, wi, 0, :],
                                     start=True, stop=False)
                    nc.tensor.matmul(pl[:], ke_sb[:, 1, :], khT[:, wi, 1, :],
                                     start=False, stop=False)
                    # per-head (kh + qe)^T @ qs term (folds the q_emb term)
                    for h in range(H):
                        half = (h % 2) * QK
                        cb = h * QK
                        for xt in range(XT):
                            nc.tensor.matmul(
                                pl[half:half + QK, cb:cb + QK],
                                khq[:, wi, xt, cb:cb + QK],
                                qsT[:, wi, xt, cb:cb + QK],
                                start=False, stop=(h == H - 1 and xt == XT - 1),
                                tile_position=(0, half))
                    e_t = mid.tile([128, C], bf16, tag="e")
                    nc.scalar.activation(e_t[:], pl[:], AF.Exp)

                    for t in range(KT):          # head pairs (2t, 2t+1)
                        pu = psU.tile([128, X + 2], f32, tag="pu")
                        for j in range(2):       # j=0 even head, j=1 odd head
                            h = 2 * t + j
                            half = j * QK
                            nc.tensor.matmul(
                                pu[half:half + QK, :],
                                e_t[half:half + QK, h * QK:(h + 1) * QK],
                                vplus[half:half + QK, t, wi, :],
                                start=True, stop=True,
                                tile_position=(half, half))
                        recip = small.tile([128, 1], f32, tag="recip")
                        nc.vector.reciprocal(recip[:], pu[:, X:X + 1])
                        if t % 2 == 0:
                            nc.scalar.activation(
                                attn[:, t, wi, :],
                                pu[:, 0:X], AF.Copy, scale=recip[:])
                        else:
                            nc.vector.tensor_scalar_mul(
                                attn[:, t, wi, :], pu[:, 0:X], recip[:])

                # --- output projection ---
                for ot in range(KT):
                    po = psVL.tile([128, 2, X], f32, tag="vl")
                    for kt in range(KT):
                        nc.tensor.matmul(
                            po[:],
                            wo_sb[:, kt, ot * 128:(ot + 1) * 128],
                            attn[:, kt, :, :],
                            start=(kt == 0), stop=(kt == KT - 1))
                    ob = mid.tile([128, 2, X], f32, tag="ob")
                    nc.scalar.activation(ob[:, 0, :], po[:, 0, :], AF.Copy)
                    nc.vector.tensor_copy(ob[:, 1, :], po[:, 1, :])
                    nc.sync.dma_start(
                        out[ot * 128:(ot + 1) * 128, w0:w0 + 2, :], ob[:])

    nc.compile()
    return nc


def _get_program():
    if "nc" not in _CACHE:
        _CACHE["nc"] = _build_program()
    return _CACHE["nc"]


def _make_in_maps(query, key_, value, Wq, Wk, Wv, Wo, q_emb, k_emb, v_emb):
    import ml_dtypes
    bf16 = ml_dtypes.bfloat16
    scale = np.float32(1.0 / np.sqrt(X))
    wqt = np.ascontiguousarray((Wq.T * scale).astype(bf16))
    wkt = np.ascontiguousarray(Wk.T.astype(bf16))
    wvt = np.ascontiguousarray(Wv.T.astype(bf16))
    wot = np.ascontiguousarray(Wo.T.astype(bf16))
    qe8 = np.ascontiguousarray(np.tile(q_emb, (1, H)).astype(bf16))
    ke2 = np.ascontiguousarray(np.concatenate([k_emb, k_emb], axis=1).astype(bf16))
    vet = np.ascontiguousarray(v_emb.T)
    def shard(a, ws):
        # (C, X, WC) -> [pair, i, w, x] contiguous, bf16
        return np.ascontiguousarray(
            a[:, :, ws].reshape(C, X, PAIRS, 2).transpose(2, 0, 3, 1).astype(bf16))

    in_maps = []
    for c in range(N_CORES):
        ws = slice(c * WC, (c + 1) * WC)
        in_maps.append({
            "qin": shard(query, ws),
            "kin": shard(key_, ws),
            "vin": shard(value, ws),
            "wqt": wqt, "wkt": wkt, "wvt": wvt, "wot": wot,
            "qe8": qe8, "ke2": ke2, "vet": vet,
            "oned": np.ones((128, 4), bf16),
        })
    return in_maps


def _run(in_maps, trace=False):
    from concourse.bass_utils import run_bass_kernel_spmd
    nc = _get_program()
    return run_bass_kernel_spmd(nc, in_maps, list(range(N_CORES)), trace=trace)


def kernel(query, key_, value, Wq, Wk, Wv, Wo, q_emb, k_emb, v_emb):
    args = (query, key_, value, Wq, Wk, Wv, Wo, q_emb, k_emb, v_emb)
    in_maps = _make_in_maps(*[np.ascontiguousarray(a, np.float32) for a in args])
    res = _run(in_maps, trace=False)
    out = np.empty((C, X, W), np.float32)
    for c in range(N_CORES):
        out[:, :, c * WC:(c + 1) * WC] = res.results[c]["out"].transpose(0, 2, 1)
    return out



# revision 2
# speedup vs baseline: 1.0553x; 1.0553x over previous
# Trainium2 Bass kernel for nn_AxialAttention (8 NeuronCores, W-parallel).
#
# Sharding: the W axis (axis=2, the vmapped axis) is split into 8 contiguous
# slices of 32 columns, one per core. Every part of the computation (the four
# 1x1-conv GEMMs, the per-(head, w) axial attention, the embedding terms) is
# independent across w, so there are no collectives; the small weight matrices
# and embedding tables are replicated to every core.
#
# Per-core math for one w column (all heads):
#   qsT[x, (h c)] = query[:, :, w].T @ (Wq.T / 16)
#   khT[x, (h c)] = key_[:, :, w].T @ Wk.T
#   vh [(h c), x] = Wv @ value[:, :, w]
#   logits_h[C, c] = khT_h.T @ qsT_h + q_emb.T @ qsT_h + k_emb.T @ khT_h
#   E = exp(logits)             (max-subtraction unnecessary: |logits| < ~2)
#   U_h = E_h.T @ [vh_h + ve | 1]          (ones column gives the softmax
#   attn_h = U_h[:, :256] / U_h[:, 256]     denominator for free)
#   out[:, :, w] = Wo @ attn
#
# Precision plan (validated numerically, absmax-rel err ~8e-3 vs 2e-2 gate):
# the q/k projections run as fp8(e4m3) DoubleRow matmuls — the 512-channel
# contraction folds into pairs of 256-deep matmuls at 2 MACs/PE-cell/cycle,
# halving the dominant GEMM cost. The softmax damps the resulting ~2.6%
# logits-path noise to ~5e-3 at the output. The v/o path (error passes 1:1
# to the output) stays bf16, as does all of the attention arithmetic.
# fp8 weight pre-scales (2x for Wq, 8x for Wk) keep the tiny 0.02-std
# weights out of the e4m3 subnormal range; the compensating 1/32 / 1/8
# lands in the PSUM-evacuation scale, so the attention sees bit-identical
# semantics to the bf16 version. Output DMA is bf16 (halves the drain tail);
# the host upcasts.

import numpy as np

H = 8          # heads
QK = 64        # per-head qk/vo channels
C = 512        # io channels
X = 256        # spatial H (attention contraction axis)
W = 256        # spatial W (vmapped axis, sharded)
N_CORES = 8
WC = W // N_CORES   # w columns per core
PAIRS = WC // 2

_CACHE = {}


def _build_program():
    import concourse.mybir as mybir
    import concourse.tile as tile
    from concourse import bacc

    f32 = mybir.dt.float32
    bf16 = mybir.dt.bfloat16
    fp8 = mybir.dt.float8e4
    AF = mybir.ActivationFunctionType
    DR = mybir.MatmulPerfMode.DoubleRow

    nc = bacc.Bacc("TRN2", target_bir_lowering=False, debug=False,
                   num_devices=N_CORES)

    # q/k inputs are fp8, laid out [pair, p, chunk, slot, wi, x] so each
    # partition's bytes are contiguous and the (chunk, slot) dims address
    # the DoubleRow 256-deep contraction: channel = chunk*256 + slot*128 + p.
    q8in = nc.dram_tensor("q8in", [PAIRS, 128, 2, 2, 2, X], fp8,
                          kind="ExternalInput").ap()
    k8in = nc.dram_tensor("k8in", [PAIRS, 128, 2, 2, 2, X], fp8,
                          kind="ExternalInput").ap()
    vin = nc.dram_tensor("vin", [PAIRS, C, 2, X], bf16, kind="ExternalInput").ap()
    wq8 = nc.dram_tensor("wq8", [128, 2, 2, C], fp8, kind="ExternalInput").ap()
    wk8 = nc.dram_tensor("wk8", [128, 2, 2, C], fp8, kind="ExternalInput").ap()
    wvt = nc.dram_tensor("wvt", [C, C], bf16, kind="ExternalInput").ap()
    wot = nc.dram_tensor("wot", [C, C], bf16, kind="ExternalInput").ap()
    qe8 = nc.dram_tensor("qe8", [X, H * QK], bf16, kind="ExternalInput").ap()
    ke2 = nc.dram_tensor("ke2", [X, 2 * QK], bf16, kind="ExternalInput").ap()
    vet = nc.dram_tensor("vet", [QK, X], f32, kind="ExternalInput").ap()
    oned = nc.dram_tensor("oned", [128, 4], bf16, kind="ExternalInput").ap()
    out = nc.dram_tensor("out", [C, WC, X], bf16, kind="ExternalOutput").ap()

    KT = C // 128   # 4 contraction tiles of the channel dim (bf16 v/o path)
    XT = X // 128   # 2 tiles of the spatial-x dim
    QS_SCALE = 1.0 / 32.0   # PSUM_q = 2*qh -> qs = qh/16
    KH_SCALE = 1.0 / 8.0    # PSUM_k = 8*kh -> kh

    with tile.TileContext(nc) as tc:
        with (
            tc.tile_pool(name="consts", bufs=1) as consts,
            tc.tile_pool(name="inp", bufs=3) as inp,
            tc.tile_pool(name="qkt", bufs=2) as qkt,
            tc.tile_pool(name="mid", bufs=2) as mid,
            tc.tile_pool(name="small", bufs=8) as small,
            tc.tile_pool(name="psA", bufs=3, space="PSUM") as psA,
            tc.tile_pool(name="psVL", bufs=2, space="PSUM") as psVL,
            tc.tile_pool(name="psU", bufs=3, space="PSUM") as psU,
        ):
            def load_inputs(pair):
                q_t = inp.tile([128, 2, 2, 2, X], fp8, tag="q_t")
                nc.sync.dma_start(
                    q_t[:], q8in[pair].rearrange("p c s w x -> p (c s w x)"))
                k_t = inp.tile([128, 2, 2, 2, X], fp8, tag="k_t")
                nc.sync.dma_start(
                    k_t[:], k8in[pair].rearrange("p c s w x -> p (c s w x)"))
                v_t = inp.tile([128, KT, 2, X], bf16, tag="v_t")
                nc.sync.dma_start(
                    v_t[:], vin[pair].rearrange("(kt p) w x -> p kt (w x)", p=128))
                return q_t, k_t, v_t

            # pair-0 inputs first so the PE can start ASAP; q is split per
            # chunk so the first matmul only waits for one half. Constants
            # go on the ACT HWDGE ring so the two DMA streams run in parallel.
            q0 = inp.tile([128, 2, 2, 2, X], fp8, tag="q_t")
            for c in range(2):
                nc.sync.dma_start(
                    q0[:, c], q8in[0, :, c].rearrange("p s w x -> p (s w x)"))
            k0 = inp.tile([128, 2, 2, 2, X], fp8, tag="k_t")
            nc.sync.dma_start(
                k0[:], k8in[0].rearrange("p c s w x -> p (c s w x)"))
            v0 = inp.tile([128, KT, 2, X], bf16, tag="v_t")
            nc.sync.dma_start(
                v0[:], vin[0].rearrange("(kt p) w x -> p kt (w x)", p=128))
            prefetched = (q0, k0, v0)

            wq_sb = consts.tile([128, 2, 2, C], fp8)
            nc.scalar.dma_start(wq_sb[:], wq8.rearrange("p c s o -> p (c s o)"))
            wk_sb = consts.tile([128, 2, 2, C], fp8)
            nc.scalar.dma_start(wk_sb[:], wk8.rearrange("p c s o -> p (c s o)"))
            wv_sb = consts.tile([128, KT, C], bf16)
            nc.scalar.dma_start(wv_sb[:], wvt.rearrange("(kt p) o -> p kt o", p=128))
            wo_sb = consts.tile([128, KT, C], bf16)
            nc.scalar.dma_start(wo_sb[:], wot.rearrange("(kt p) o -> p kt o", p=128))
            qe8_sb = consts.tile([128, XT, H * QK], bf16)
            nc.scalar.dma_start(qe8_sb[:], qe8.rearrange("(xt p) m -> p xt m", p=128))
            ke_sb = consts.tile([128, XT, 2 * QK], bf16)
            nc.scalar.dma_start(ke_sb[:], ke2.rearrange("(xt p) m -> p xt m", p=128))
            ve_sb = consts.tile([128, X], f32)
            nc.scalar.dma_start(ve_sb[0:QK, :], vet[:])
            nc.scalar.dma_start(ve_sb[QK:128, :], vet[:])
            ones_sb = consts.tile([128, 2, 2], bf16)
            nc.scalar.dma_start(ones_sb[:], oned.rearrange("p (a b) -> p a b", a=2))

            for pair in range(PAIRS):
                w0 = pair * 2
                q_t, k_t, v_t = prefetched if pair == 0 else load_inputs(pair)

                # --- q/k projections (fp8 DoubleRow), transposed layout:
                # qsT/khT [x, (h c)]; contraction 512 = 2 chunks x 256 ---
                qsT = qkt.tile([128, 2, XT, C], bf16)   # [x_p, w, xt, o]
                khT = qkt.tile([128, 2, XT, C], bf16)
                khq = qkt.tile([128, 2, XT, C], bf16)   # khT + q_emb (folds t2)
                for wi in range(2):
                    for xt in range(XT):
                        pq = psA.tile([128, C], f32, tag="mm")
                        for c in range(2):
                            nc.tensor.matmul(
                                pq[:],
                                q_t[:, c, :, wi, xt * 128:(xt + 1) * 128],
                                wq_sb[:, c, :, :],
                                start=(c == 0), stop=(c == 1),
                                perf_mode=DR)
                        nc.scalar.activation(qsT[:, wi, xt, 0:256], pq[:, 0:256],
                                             AF.Copy, scale=QS_SCALE)
                        nc.vector.tensor_scalar_mul(qsT[:, wi, xt, 256:512],
                                                    pq[:, 256:512], QS_SCALE)
                        pk = psA.tile([128, C], f32, tag="mm")
                        for c in range(2):
                            nc.tensor.matmul(
                                pk[:],
                                k_t[:, c, :, wi, xt * 128:(xt + 1) * 128],
                                wk_sb[:, c, :, :],
                                start=(c == 0), stop=(c == 1),
                                perf_mode=DR)
                        nc.vector.tensor_scalar_mul(khT[:, wi, xt, 0:256],
                                                    pk[:, 0:256], KH_SCALE)
                        nc.scalar.activation(khT[:, wi, xt, 256:512],
                                             pk[:, 256:512], AF.Copy,
                                             scale=KH_SCALE)
                        nc.gpsimd.tensor_add(khq[:, wi, xt, :],
                                             khT[:, wi, xt, :], qe8_sb[:, xt, :])

                # --- v projection + ve add + ones column (bf16) ---
                vplus = mid.tile([128, KT, 2, X + 2], bf16)  # [c2_p, head-pair, w, x+2]
                for ot in range(KT):
                    pv = psVL.tile([128, 2, X], f32, tag="vl")
                    for kt in range(KT):
                        nc.tensor.matmul(
                            pv[:],
                            wv_sb[:, kt, ot * 128:(ot + 1) * 128],
                            v_t[:, kt, :, :],
                            start=(kt == 0), stop=(kt == KT - 1))
                    for wi in range(2):
                        nc.vector.tensor_add(
                            vplus[:, ot, wi, 0:X], pv[:, wi, :], ve_sb[:])
                    nc.vector.tensor_copy(vplus[:, ot, :, X:X + 2], ones_sb[:])

                # --- per-w attention (bf16) ---
                attn = mid.tile([128, KT, 2, X], bf16)  # [(h c)_p, kt, w, x]
                for wi in range(2):
                    pl = psVL.tile([128, C], f32, tag="vl")
                    # k_emb term, all heads at once (dup'd table)
                    nc.tensor.matmul(pl[:], ke_sb[:, 0, :], khT[:, wi, 0, :],
                                     start=True, stop=False)
                    nc.tensor.matmul(pl[:], ke_sb[:, 1, :], khT[:, wi, 1, :],
                                     start=False, stop=False)
                    # per-head (kh + qe)^T @ qs term (folds the q_emb term)
                    for h in range(H):
                        half = (h % 2) * QK
                        cb = h * QK
                        for xt in range(XT):
                            nc.tensor.matmul(
                                pl[half:half + QK, cb:cb + QK],
                                khq[:, wi, xt, cb:cb + QK],
                                qsT[:, wi, xt, cb:cb + QK],
                                start=False, stop=(h == H - 1 and xt == XT - 1),
                                tile_position=(0, half))
                    e_t = mid.tile([128, C], bf16, tag="e")
                    nc.scalar.activation(e_t[:], pl[:], AF.Exp)

                    for t in range(KT):          # head pairs (2t, 2t+1)
                        pu = psU.tile([128, X + 2], f32, tag="pu")
                        for j in range(2):       # j=0 even head, j=1 odd head
                            h = 2 * t + j
                            half = j * QK
                            nc.tensor.matmul(
                                pu[half:half + QK, :],
                                e_t[half:half + QK, h * QK:(h + 1) * QK],
                                vplus[half:half + QK, t, wi, :],
                                start=True, stop=True,
                                tile_position=(half, half))
                        recip = small.tile([128, 1], f32, tag="recip")
                        nc.vector.reciprocal(recip[:], pu[:, X:X + 1])
                        if t % 2 == 0:
                            nc.scalar.activation(
                                attn[:, t, wi, :],
                                pu[:, 0:X], AF.Copy, scale=recip[:])
                        else:
                            nc.vector.tensor_scalar_mul(
                                attn[:, t, wi, :], pu[:, 0:X], recip[:])

                # --- output projection (bf16), bf16 DMA out ---
                for ot in range(KT):
                    po = psVL.tile([128, 2, X], f32, tag="vl")
                    for kt in range(KT):
                        nc.tensor.matmul(
                            po[:],
                            wo_sb[:, kt, ot * 128:(ot + 1) * 128],
                            attn[:, kt, :, :],
                            start=(kt == 0), stop=(kt == KT - 1))
                    ob = mid.tile([128, 2, X], bf16, tag="ob")
                    nc.scalar.activation(ob[:, 0, :], po[:, 0, :], AF.Copy)
                    nc.vector.tensor_copy(ob[:, 1, :], po[:, 1, :])
                    nc.sync.dma_start(
                        out[ot * 128:(ot + 1) * 128, w0:w0 + 2, :], ob[:])

    nc.compile()
    return nc


def _get_program():
    if "nc" not in _CACHE:
        _CACHE["nc"] = _build_program()
    return _CACHE["nc"]


def _make_in_maps(query, key_, value, Wq, Wk, Wv, Wo, q_emb, k_emb, v_emb):
    import ml_dtypes
    bf16 = ml_dtypes.bfloat16
    fp8 = ml_dtypes.float8_e4m3

    # fp8 weights: pre-scaled (2x / 8x) to clear the e4m3 subnormal floor;
    # compensated by the PSUM-evacuation scales inside the kernel.
    # layout [p, chunk, slot, o] with channel = chunk*256 + slot*128 + p.
    def w8(Wm, s):
        return np.ascontiguousarray(
            (Wm.T * s).reshape(2, 2, 128, C).transpose(2, 0, 1, 3).astype(fp8))

    wq8 = w8(Wq, 2.0)
    wk8 = w8(Wk, 8.0)
    wvt = np.ascontiguousarray(Wv.T.astype(bf16))
    wot = np.ascontiguousarray(Wo.T.astype(bf16))
    qe8 = np.ascontiguousarray(np.tile(q_emb, (1, H)).astype(bf16))
    ke2 = np.ascontiguousarray(np.concatenate([k_emb, k_emb], axis=1).astype(bf16))
    vet = np.ascontiguousarray(v_emb.T)

    def shard8(a, ws):
        # (C, X, WC) -> [pair, p, chunk, slot, wi, x] contiguous fp8
        return np.ascontiguousarray(
            a[:, :, ws].reshape(2, 2, 128, X, PAIRS, 2)
            .transpose(4, 2, 0, 1, 5, 3).astype(fp8))

    def shardv(a, ws):
        # (C, X, WC) -> [pair, i, w, x] contiguous bf16
        return np.ascontiguousarray(
            a[:, :, ws].reshape(C, X, PAIRS, 2).transpose(2, 0, 3, 1).astype(bf16))

    in_maps = []
    for c in range(N_CORES):
        ws = slice(c * WC, (c + 1) * WC)
        in_maps.append({
            "q8in": shard8(query, ws),
            "k8in": shard8(key_, ws),
            "vin": shardv(value, ws),
            "wq8": wq8, "wk8": wk8, "wvt": wvt, "wot": wot,
            "qe8": qe8, "ke2": ke2, "vet": vet,
            "oned": np.ones((128, 4), bf16),
        })
    return in_maps


def _run(in_maps, trace=False):
    from concourse.bass_utils import run_bass_kernel_spmd
    nc = _get_program()
    return run_bass_kernel_spmd(nc, in_maps, list(range(N_CORES)), trace=trace)


def kernel(query, key_, value, Wq, Wk, Wv, Wo, q_emb, k_emb, v_emb):
    args = (query, key_, value, Wq, Wk, Wv, Wo, q_emb, k_emb, v_emb)
    in_maps = _make_in_maps(*[np.ascontiguousarray(a, np.float32) for a in args])
    res = _run(in_maps, trace=False)
    out = np.empty((C, X, W), np.float32)
    for c in range(N_CORES):
        out[:, :, c * WC:(c + 1) * WC] = (
            res.results[c]["out"].astype(np.float32).transpose(0, 2, 1))
    return out


# revision 3
# speedup vs baseline: 1.1213x; 1.0625x over previous
# Trainium2 Bass kernel for nn_AxialAttention (8 NeuronCores, W-parallel).
#
# Sharding: the W axis (axis=2, the vmapped axis) is split into 8 contiguous
# slices of 32 columns, one per core; all weights/tables are replicated.
# No collectives.
#
# Per-core math for one w column (all heads):
#   qsT[x, (h c)] = query[:, :, w].T @ (Wq.T / 16)
#   khT[x, (h c)] = key_[:, :, w].T @ Wk.T
#   vh [(h c), x] = Wv @ value[:, :, w]
#   logits_h[C, c] = khT_h.T @ qsT_h + q_emb.T @ qsT_h + k_emb.T @ khT_h
#   E = exp(logits)             (max-subtraction unnecessary: |logits| < ~2)
#   U_h = E_h.T @ [vh_h + ve | 1]          (ones column gives the softmax
#   attn_h = U_h[:, :256] / U_h[:, 256]     denominator for free)
#   out[:, :, w] = Wo @ attn
#
# Precision plan (validated numerically, absmax-rel err ~8e-3 vs 2e-2 gate):
# the q/k projections run as fp8(e4m3) DoubleRow matmuls — the 512-channel
# contraction folds into pairs of 256-deep matmuls at 2 MACs/PE-cell/cycle,
# halving the dominant GEMM cost. The softmax damps the resulting logits-path
# noise to ~5e-3 at the output. The v/o path (error passes 1:1 to the output)
# stays bf16, as does the attention arithmetic. fp8 weight pre-scales (2x Wq,
# 8x Wk) clear the e4m3 subnormal floor; the compensating 1/32 and 1/8 are
# folded into the PSUM-evacuation scales. Output DMA is bf16 (host upcasts).
#
# Queue layout (avoids head-of-line blocking measured in earlier traces):
# the Sync ring carries ONLY input DMAs, issued one pair ahead of use; the
# batched per-pair output DMA lives on the ACT ring directly after the
# (all-scalar) PSUM evacuations it depends on, so no ring ever parks on a
# cross-engine semaphore while later descriptors queue behind it.

import numpy as np

H = 8          # heads
QK = 64        # per-head qk/vo channels
C = 512        # io channels
X = 256        # spatial H (attention contraction axis)
W = 256        # spatial W (vmapped axis, sharded)
N_CORES = 8
WC = W // N_CORES   # w columns per core
PAIRS = WC // 2

_CACHE = {}


def _build_program():
    import concourse.mybir as mybir
    import concourse.tile as tile
    from concourse import bacc

    f32 = mybir.dt.float32
    bf16 = mybir.dt.bfloat16
    fp8 = mybir.dt.float8e4
    AF = mybir.ActivationFunctionType
    DR = mybir.MatmulPerfMode.DoubleRow

    nc = bacc.Bacc("TRN2", target_bir_lowering=False, debug=False,
                   num_devices=N_CORES)

    # q/k packed fp8 input, [pair, p, qk, chunk, slot, wi, x]: channel =
    # chunk*256 + slot*128 + p; per-partition bytes contiguous (4 KiB).
    qk8in = nc.dram_tensor("qk8in", [PAIRS, 128, 2, 2, 2, 2, X], fp8,
                           kind="ExternalInput").ap()
    vin = nc.dram_tensor("vin", [PAIRS, C, 2, X], bf16, kind="ExternalInput").ap()
    wq8 = nc.dram_tensor("wq8", [128, 2, 2, C], fp8, kind="ExternalInput").ap()
    wk8 = nc.dram_tensor("wk8", [128, 2, 2, C], fp8, kind="ExternalInput").ap()
    wvt = nc.dram_tensor("wvt", [C, C], bf16, kind="ExternalInput").ap()
    wot = nc.dram_tensor("wot", [C, C], bf16, kind="ExternalInput").ap()
    qe8 = nc.dram_tensor("qe8", [X, H * QK], bf16, kind="ExternalInput").ap()
    ke2 = nc.dram_tensor("ke2", [X, 2 * QK], bf16, kind="ExternalInput").ap()
    vet = nc.dram_tensor("vet", [QK, X], f32, kind="ExternalInput").ap()
    oned = nc.dram_tensor("oned", [128, 4], bf16, kind="ExternalInput").ap()
    out = nc.dram_tensor("out", [C, WC, X], bf16, kind="ExternalOutput").ap()

    KT = C // 128   # 4 contraction tiles of the channel dim (bf16 v/o path)
    XT = X // 128   # 2 tiles of the spatial-x dim
    QS_SCALE = 1.0 / 32.0   # PSUM_q = 2*qh -> qs = qh/16
    KH_SCALE = 1.0 / 8.0    # PSUM_k = 8*kh -> kh

    with tile.TileContext(nc) as tc:
        with (
            tc.tile_pool(name="consts", bufs=1) as consts,
            tc.tile_pool(name="inp", bufs=4) as inp,
            tc.tile_pool(name="qkt", bufs=2) as qkt,
            tc.tile_pool(name="mid", bufs=2) as mid,
            tc.tile_pool(name="small", bufs=8) as small,
            tc.tile_pool(name="psA", bufs=3, space="PSUM") as psA,
            tc.tile_pool(name="psVL", bufs=2, space="PSUM") as psVL,
            tc.tile_pool(name="psU", bufs=3, space="PSUM") as psU,
        ):
            def load_inputs(pair):
                qk_t = inp.tile([128, 2, 2, 2, 2, X], fp8, tag="qk_t")
                nc.sync.dma_start(
                    qk_t[:], qk8in[pair].rearrange("p q c s w x -> p (q c s w x)"))
                v_t = inp.tile([128, KT, 2, X], bf16, tag="v_t")
                nc.sync.dma_start(
                    v_t[:], vin[pair].rearrange("(kt p) w x -> p kt (w x)", p=128))
                return qk_t, v_t

            # pair-0 inputs first so the PE can start ASAP; q is split per
            # chunk so the first matmul group only waits for 128 KiB.
            qk0 = inp.tile([128, 2, 2, 2, 2, X], fp8, tag="qk_t")
            for c in range(2):
                nc.sync.dma_start(
                    qk0[:, 0, c], qk8in[0, :, 0, c].rearrange("p s w x -> p (s w x)"))
            nc.sync.dma_start(
                qk0[:, 1], qk8in[0, :, 1].rearrange("p c s w x -> p (c s w x)"))
            v0 = inp.tile([128, KT, 2, X], bf16, tag="v_t")
            nc.sync.dma_start(
                v0[:], vin[0].rearrange("(kt p) w x -> p kt (w x)", p=128))
            prefetched = (qk0, v0)

            wq_sb = consts.tile([128, 2, 2, C], fp8)
            nc.scalar.dma_start(wq_sb[:], wq8.rearrange("p c s o -> p (c s o)"))
            wk_sb = consts.tile([128, 2, 2, C], fp8)
            nc.scalar.dma_start(wk_sb[:], wk8.rearrange("p c s o -> p (c s o)"))
            wv_sb = consts.tile([128, KT, C], bf16)
            nc.scalar.dma_start(wv_sb[:], wvt.rearrange("(kt p) o -> p kt o", p=128))
            wo_sb = consts.tile([128, KT, C], bf16)
            nc.scalar.dma_start(wo_sb[:], wot.rearrange("(kt p) o -> p kt o", p=128))
            qe8_sb = consts.tile([128, XT, H * QK], bf16)
            nc.scalar.dma_start(qe8_sb[:], qe8.rearrange("(xt p) m -> p xt m", p=128))
            ke_sb = consts.tile([128, XT, 2 * QK], bf16)
            nc.scalar.dma_start(ke_sb[:], ke2.rearrange("(xt p) m -> p xt m", p=128))
            ve_sb = consts.tile([128, X], f32)
            nc.scalar.dma_start(ve_sb[0:QK, :], vet[:])
            nc.scalar.dma_start(ve_sb[QK:128, :], vet[:])
            ones_sb = consts.tile([128, 2, 2], bf16)
            nc.scalar.dma_start(ones_sb[:], oned.rearrange("p (a b) -> p a b", a=2))

            for pair in range(PAIRS):
                w0 = pair * 2
                qk_t, v_t = prefetched if pair == 0 else prefetched_next

                # --- q/k projections (fp8 DoubleRow), transposed layout:
                # qsT/khT [x, (h c)]; contraction 512 = 2 chunks x 256 ---
                qsT = qkt.tile([128, 2, XT, C], bf16)   # [x_p, w, xt, o]
                khT = qkt.tile([128, 2, XT, C], bf16)
                khq = qkt.tile([128, 2, XT, C], bf16)   # khT + q_emb (folds t2)
                for wi in range(2):
                    for xt in range(XT):
                        pq = psA.tile([128, C], f32, tag="mm")
                        for c in range(2):
                            nc.tensor.matmul(
                                pq[:],
                                qk_t[:, 0, c, :, wi, xt * 128:(xt + 1) * 128],
                                wq_sb[:, c, :, :],
                                start=(c == 0), stop=(c == 1),
                                perf_mode=DR)
                        nc.scalar.activation(qsT[:, wi, xt, 0:256], pq[:, 0:256],
                                             AF.Copy, scale=QS_SCALE)
                        nc.vector.tensor_scalar_mul(qsT[:, wi, xt, 256:512],
                                                    pq[:, 256:512], QS_SCALE)
                        pk = psA.tile([128, C], f32, tag="mm")
                        for c in range(2):
                            nc.tensor.matmul(
                                pk[:],
                                qk_t[:, 1, c, :, wi, xt * 128:(xt + 1) * 128],
                                wk_sb[:, c, :, :],
                                start=(c == 0), stop=(c == 1),
                                perf_mode=DR)
                        nc.vector.tensor_scalar_mul(khT[:, wi, xt, 0:256],
                                                    pk[:, 0:256], KH_SCALE)
                        nc.scalar.activation(khT[:, wi, xt, 256:512],
                                             pk[:, 256:512], AF.Copy,
                                             scale=KH_SCALE)
                        nc.gpsimd.tensor_add(khq[:, wi, xt, :],
                                             khT[:, wi, xt, :], qe8_sb[:, xt, :])

                # --- v projection + ve add + ones column (bf16) ---
                vplus = mid.tile([128, KT, 2, X + 2], bf16)  # [c2_p, head-pair, w, x+2]
                for ot in range(KT):
                    pv = psVL.tile([128, 2, X], f32, tag="vl")
                    for kt in range(KT):
                        nc.tensor.matmul(
                            pv[:],
                            wv_sb[:, kt, ot * 128:(ot + 1) * 128],
                            v_t[:, kt, :, :],
                            start=(kt == 0), stop=(kt == KT - 1))
                    for wi in range(2):
                        nc.vector.tensor_add(
                            vplus[:, ot, wi, 0:X], pv[:, wi, :], ve_sb[:])
                    nc.vector.tensor_copy(vplus[:, ot, :, X:X + 2], ones_sb[:])

                # prefetch the next pair's inputs HERE so their descriptors
                # sit ahead of everything the tail of this pair produces.
                if pair + 1 < PAIRS:
                    prefetched_next = load_inputs(pair + 1)

                # --- per-w attention (bf16) ---
                attn = mid.tile([128, KT, 2, X], bf16)  # [(h c)_p, kt, w, x]
                for wi in range(2):
                    pl = psVL.tile([128, C], f32, tag="vl")
                    # k_emb term, all heads at once (dup'd table)
                    nc.tensor.matmul(pl[:], ke_sb[:, 0, :], khT[:, wi, 0, :],
                                     start=True, stop=False)
                    nc.tensor.matmul(pl[:], ke_sb[:, 1, :], khT[:, wi, 1, :],
                                     start=False, stop=False)
                    # per-head (kh + qe)^T @ qs term (folds the q_emb term)
                    for h in range(H):
                        half = (h % 2) * QK
                        cb = h * QK
                        for xt in range(XT):
                            nc.tensor.matmul(
                                pl[half:half + QK, cb:cb + QK],
                                khq[:, wi, xt, cb:cb + QK],
                                qsT[:, wi, xt, cb:cb + QK],
                                start=False, stop=(h == H - 1 and xt == XT - 1),
                                tile_position=(0, half))
                    e_t = mid.tile([128, C], bf16, tag="e")
                    nc.scalar.activation(e_t[:], pl[:], AF.Exp)

                    for t in range(KT):          # head pairs (2t, 2t+1)
                        pu = psU.tile([128, X + 2], f32, tag="pu")
                        for j in range(2):       # j=0 even head, j=1 odd head
                            h = 2 * t + j
                            half = j * QK
                            nc.tensor.matmul(
                                pu[half:half + QK, :],
                                e_t[half:half + QK, h * QK:(h + 1) * QK],
                                vplus[half:half + QK, t, wi, :],
                                start=True, stop=True,
                                tile_position=(half, half))
                        recip = small.tile([128, 1], f32, tag="recip")
                        nc.vector.reciprocal(recip[:], pu[:, X:X + 1])
                        nc.vector.tensor_scalar_mul(
                            attn[:, t, wi, :], pu[:, 0:X], recip[:])

                # --- output projection (bf16); batched bf16 DMA out on the
                # ACT ring right behind its all-scalar evacuations ---
                ob = mid.tile([128, KT, 2, X], bf16, tag="ob")
                for ot in range(KT):
                    po = psVL.tile([128, 2, X], f32, tag="vl")
                    for kt in range(KT):
                        nc.tensor.matmul(
                            po[:],
                            wo_sb[:, kt, ot * 128:(ot + 1) * 128],
                            attn[:, kt, :, :],
                            start=(kt == 0), stop=(kt == KT - 1))
                    nc.scalar.activation(ob[:, ot], po[:], AF.Copy)
                nc.scalar.dma_start(
                    out[:, w0:w0 + 2, :].rearrange("(ot p) w x -> p ot (w x)",
                                                   p=128),
                    ob[:])

    nc.compile()
    return nc


def _get_program():
    if "nc" not in _CACHE:
        _CACHE["nc"] = _build_program()
    return _CACHE["nc"]


def _make_in_maps(query, key_, value, Wq, Wk, Wv, Wo, q_emb, k_emb, v_emb):
    import ml_dtypes
    bf16 = ml_dtypes.bfloat16
    fp8 = ml_dtypes.float8_e4m3

    # fp8 weights: pre-scaled (2x / 8x) to clear the e4m3 subnormal floor;
    # compensated by the PSUM-evacuation scales inside the kernel.
    # layout [p, chunk, slot, o] with channel = chunk*256 + slot*128 + p.
    def w8(Wm, s):
        return np.ascontiguousarray(
            (Wm.T * s).reshape(2, 2, 128, C).transpose(2, 0, 1, 3).astype(fp8))

    wq8 = w8(Wq, 2.0)
    wk8 = w8(Wk, 8.0)
    wvt = np.ascontiguousarray(Wv.T.astype(bf16))
    wot = np.ascontiguousarray(Wo.T.astype(bf16))
    qe8 = np.ascontiguousarray(np.tile(q_emb, (1, H)).astype(bf16))
    ke2 = np.ascontiguousarray(np.concatenate([k_emb, k_emb], axis=1).astype(bf16))
    vet = np.ascontiguousarray(v_emb.T)

    def shardv(a, ws):
        # (C, X, WC) -> [pair, i, w, x] contiguous bf16
        return np.ascontiguousarray(
            a[:, :, ws].reshape(C, X, PAIRS, 2).transpose(2, 0, 3, 1).astype(bf16))

    in_maps = []
    qk8s = {}
    for c in range(N_CORES):
        ws = slice(c * WC, (c + 1) * WC)
        # (C, X, WC) x2 -> [pair, p, qk, chunk, slot, wi, x] contiguous fp8
        qk = np.stack([query[:, :, ws], key_[:, :, ws]])  # (2, C, X, WC)
        qk8s[c] = np.ascontiguousarray(
            qk.reshape(2, 2, 2, 128, X, PAIRS, 2)
            .transpose(5, 3, 0, 1, 2, 6, 4).astype(fp8))
        in_maps.append({
            "qk8in": qk8s[c],
            "vin": shardv(value, ws),
            "wq8": wq8, "wk8": wk8, "wvt": wvt, "wot": wot,
            "qe8": qe8, "ke2": ke2, "vet": vet,
            "oned": np.ones((128, 4), bf16),
        })
    return in_maps


def _run(in_maps, trace=False):
    from concourse.bass_utils import run_bass_kernel_spmd
    nc = _get_program()
    return run_bass_kernel_spmd(nc, in_maps, list(range(N_CORES)), trace=trace)


def kernel(query, key_, value, Wq, Wk, Wv, Wo, q_emb, k_emb, v_emb):
    args = (query, key_, value, Wq, Wk, Wv, Wo, q_emb, k_emb, v_emb)
    in_maps = _make_in_maps(*[np.ascontiguousarray(a, np.float32) for a in args])
    res = _run(in_maps, trace=False)
    out = np.empty((C, X, W), np.float32)
    for c in range(N_CORES):
        out[:, :, c * WC:(c + 1) * WC] = (
            res.results[c]["out"].astype(np.float32).transpose(0, 2, 1))
    return out


# revision 4
# speedup vs baseline: 1.2098x; 1.0790x over previous
# Trainium2 Bass kernel for nn_AxialAttention (8 NeuronCores, W-parallel).
#
# Sharding: the W axis (axis=2, the vmapped axis) is split into 8 contiguous
# slices of 32 columns, one per core; all weights/tables are replicated.
# No collectives.
#
# Per-core math for one w column (all heads):
#   qsT[x, (h c)] = query[:, :, w].T @ (Wq.T / 16)
#   khT[x, (h c)] = key_[:, :, w].T @ Wk.T
#   vh [(h c), x] = Wv @ value[:, :, w]
#   logits_h[C, c] = khT_h.T @ qsT_h + q_emb.T @ qsT_h + k_emb.T @ khT_h
#   E = exp(logits)             (max-subtraction unnecessary: |logits| < ~2)
#   U_h = E_h.T @ [vh_h + ve | 1]          (ones column gives the softmax
#   attn_h = U_h[:, :256] / U_h[:, 256]     denominator for free)
#   out[:, :, w] = Wo @ attn
#
# Precision plan (validated numerically, absmax-rel err ~8e-3 vs 2e-2 gate):
# the q/k projections run as fp8(e4m3) DoubleRow matmuls — the 512-channel
# contraction folds into pairs of 256-deep matmuls at 2 MACs/PE-cell/cycle,
# halving the dominant GEMM cost. The softmax damps the resulting logits-path
# noise to ~5e-3 at the output. The v/o path (error passes 1:1 to the output)
# stays bf16, as does the attention arithmetic. fp8 weight pre-scales (2x Wq,
# 8x Wk) clear the e4m3 subnormal floor; the compensating 1/32 and 1/8 are
# folded into the PSUM-evacuation scales. Output DMA is bf16 (host upcasts).
#
# Pipeline structure: each phase runs [qk-proj(p) | logits+AV(p-1) |
# v-proj(p) | o-proj(p-1)], i.e. the attention of a pair executes one phase
# after its projections. This keeps the in-order LDWEIGHTS queue free of
# head-of-line blocking (projection weight loads depend only on input DMA
# that completed a full phase earlier, never on exp/evac chains) and gives
# every PSUM evacuation a full phase of slack. Input DMAs own the Sync ring
# and are issued a phase ahead; the batched per-pair output DMA sits on the
# ACT ring directly behind its all-scalar evacuations.

import numpy as np

H = 8          # heads
QK = 64        # per-head qk/vo channels
C = 512        # io channels
X = 256        # spatial H (attention contraction axis)
W = 256        # spatial W (vmapped axis, sharded)
N_CORES = 8
WC = W // N_CORES   # w columns per core
PAIRS = WC // 2

_CACHE = {}


def _build_program():
    import concourse.mybir as mybir
    import concourse.tile as tile
    from concourse import bacc

    f32 = mybir.dt.float32
    bf16 = mybir.dt.bfloat16
    fp8 = mybir.dt.float8e4
    AF = mybir.ActivationFunctionType
    DR = mybir.MatmulPerfMode.DoubleRow

    nc = bacc.Bacc("TRN2", target_bir_lowering=False, debug=False,
                   num_devices=N_CORES)

    # q/k packed fp8 input, [pair, p, qk, chunk, slot, wi, x]: channel =
    # chunk*256 + slot*128 + p; per-partition bytes contiguous (4 KiB).
    qk8in = nc.dram_tensor("qk8in", [PAIRS, 128, 2, 2, 2, 2, X], fp8,
                           kind="ExternalInput").ap()
    vin = nc.dram_tensor("vin", [PAIRS, C, 2, X], bf16, kind="ExternalInput").ap()
    wq8 = nc.dram_tensor("wq8", [128, 2, 2, C], fp8, kind="ExternalInput").ap()
    wk8 = nc.dram_tensor("wk8", [128, 2, 2, C], fp8, kind="ExternalInput").ap()
    wvt = nc.dram_tensor("wvt", [C, C], bf16, kind="ExternalInput").ap()
    wot = nc.dram_tensor("wot", [C, C], bf16, kind="ExternalInput").ap()
    qe8 = nc.dram_tensor("qe8", [X, H * QK], bf16, kind="ExternalInput").ap()
    ke2 = nc.dram_tensor("ke2", [X, 2 * QK], bf16, kind="ExternalInput").ap()
    vet = nc.dram_tensor("vet", [QK, X], f32, kind="ExternalInput").ap()
    oned = nc.dram_tensor("oned", [128, 4], bf16, kind="ExternalInput").ap()
    out = nc.dram_tensor("out", [C, WC, X], bf16, kind="ExternalOutput").ap()

    KT = C // 128   # 4 contraction tiles of the channel dim (bf16 v/o path)
    XT = X // 128   # 2 tiles of the spatial-x dim
    QS_SCALE = 1.0 / 32.0   # PSUM_q = 2*qh -> qs = qh/16
    KH_SCALE = 1.0 / 8.0    # PSUM_k = 8*kh -> kh

    with tile.TileContext(nc) as tc:
        with (
            tc.tile_pool(name="consts", bufs=1) as consts,
            tc.tile_pool(name="inp", bufs=4) as inp,
            tc.tile_pool(name="qkt", bufs=2) as qkt,
            tc.tile_pool(name="mid", bufs=2) as mid,
            tc.tile_pool(name="small", bufs=8) as small,
            tc.tile_pool(name="psA", bufs=3, space="PSUM") as psA,
            tc.tile_pool(name="psVL", bufs=2, space="PSUM") as psVL,
            tc.tile_pool(name="psU", bufs=3, space="PSUM") as psU,
        ):
            def load_inputs(pair):
                qk_t = inp.tile([128, 2, 2, 2, 2, X], fp8, tag="qk_t")
                nc.sync.dma_start(
                    qk_t[:], qk8in[pair].rearrange("p q c s w x -> p (q c s w x)"))
                v_t = inp.tile([128, KT, 2, X], bf16, tag="v_t")
                nc.sync.dma_start(
                    v_t[:], vin[pair].rearrange("(kt p) w x -> p kt (w x)", p=128))
                return qk_t, v_t

            # pair-0 inputs first so the PE can start ASAP; q is split per
            # chunk so the first matmul group only waits for 128 KiB.
            qk0 = inp.tile([128, 2, 2, 2, 2, X], fp8, tag="qk_t")
            for c in range(2):
                nc.sync.dma_start(
                    qk0[:, 0, c], qk8in[0, :, 0, c].rearrange("p s w x -> p (s w x)"))
            nc.sync.dma_start(
                qk0[:, 1], qk8in[0, :, 1].rearrange("p c s w x -> p (c s w x)"))
            v0 = inp.tile([128, KT, 2, X], bf16, tag="v_t")
            nc.sync.dma_start(
                v0[:], vin[0].rearrange("(kt p) w x -> p kt (w x)", p=128))

            wq_sb = consts.tile([128, 2, 2, C], fp8)
            nc.scalar.dma_start(wq_sb[:], wq8.rearrange("p c s o -> p (c s o)"))
            wk_sb = consts.tile([128, 2, 2, C], fp8)
            nc.scalar.dma_start(wk_sb[:], wk8.rearrange("p c s o -> p (c s o)"))
            wv_sb = consts.tile([128, KT, C], bf16)
            nc.scalar.dma_start(wv_sb[:], wvt.rearrange("(kt p) o -> p kt o", p=128))
            wo_sb = consts.tile([128, KT, C], bf16)
            nc.scalar.dma_start(wo_sb[:], wot.rearrange("(kt p) o -> p kt o", p=128))
            qe8_sb = consts.tile([128, XT, H * QK], bf16)
            nc.scalar.dma_start(qe8_sb[:], qe8.rearrange("(xt p) m -> p xt m", p=128))
            ke_sb = consts.tile([128, XT, 2 * QK], bf16)
            nc.scalar.dma_start(ke_sb[:], ke2.rearrange("(xt p) m -> p xt m", p=128))
            ve_sb = consts.tile([128, X], f32)
            nc.scalar.dma_start(ve_sb[0:QK, :], vet[:])
            nc.scalar.dma_start(ve_sb[QK:128, :], vet[:])
            ones_sb = consts.tile([128, 2, 2], bf16)
            nc.scalar.dma_start(ones_sb[:], oned.rearrange("p (a b) -> p a b", a=2))

            cur = {}    # live tiles for the in-flight pair (produced -> consumed
            prev = {}   # next phase): qsT/khT/khq/vplus

            def qk_proj(qk_t):
                qsT = qkt.tile([128, 2, XT, C], bf16, tag="qsT")  # [x_p, w, xt, o]
                khT = qkt.tile([128, 2, XT, C], bf16, tag="khT")
                khq = qkt.tile([128, 2, XT, C], bf16, tag="khq")  # khT + q_emb
                for wi in range(2):
                    for xt in range(XT):
                        pq = psA.tile([128, C], f32, tag="mm")
                        for c in range(2):
                            nc.tensor.matmul(
                                pq[:],
                                qk_t[:, 0, c, :, wi, xt * 128:(xt + 1) * 128],
                                wq_sb[:, c, :, :],
                                start=(c == 0), stop=(c == 1),
                                perf_mode=DR)
                        nc.scalar.activation(qsT[:, wi, xt, 0:256], pq[:, 0:256],
                                             AF.Copy, scale=QS_SCALE)
                        nc.vector.tensor_scalar_mul(qsT[:, wi, xt, 256:512],
                                                    pq[:, 256:512], QS_SCALE)
                        pk = psA.tile([128, C], f32, tag="mm")
                        for c in range(2):
                            nc.tensor.matmul(
                                pk[:],
                                qk_t[:, 1, c, :, wi, xt * 128:(xt + 1) * 128],
                                wk_sb[:, c, :, :],
                                start=(c == 0), stop=(c == 1),
                                perf_mode=DR)
                        nc.vector.tensor_scalar_mul(khT[:, wi, xt, 0:256],
                                                    pk[:, 0:256], KH_SCALE)
                        nc.scalar.activation(khT[:, wi, xt, 256:512],
                                             pk[:, 256:512], AF.Copy,
                                             scale=KH_SCALE)
                        nc.gpsimd.tensor_add(khq[:, wi, xt, :],
                                             khT[:, wi, xt, :], qe8_sb[:, xt, :])
                return qsT, khT, khq

            def v_proj(v_t):
                vplus = mid.tile([128, KT, 2, X + 2], bf16, tag="vp")
                for ot in range(KT):
                    pv = psVL.tile([128, 2, X], f32, tag="vl")
                    for kt in range(KT):
                        nc.tensor.matmul(
                            pv[:],
                            wv_sb[:, kt, ot * 128:(ot + 1) * 128],
                            v_t[:, kt, :, :],
                            start=(kt == 0), stop=(kt == KT - 1))
                    for wi in range(2):
                        nc.vector.tensor_add(
                            vplus[:, ot, wi, 0:X], pv[:, wi, :], ve_sb[:])
                    nc.vector.tensor_copy(vplus[:, ot, :, X:X + 2], ones_sb[:])
                return vplus

            def attention(qsT, khT, khq, vplus):
                # both logits groups first (exp of wi overlaps logits of wi+1
                # and the AV matmuls), then both AV groups
                pls, ets = [], []
                for wi in range(2):
                    pl = psA.tile([128, C], f32, tag="mm")
                    nc.tensor.matmul(pl[:], ke_sb[:, 0, :], khT[:, wi, 0, :],
                                     start=True, stop=False)
                    nc.tensor.matmul(pl[:], ke_sb[:, 1, :], khT[:, wi, 1, :],
                                     start=False, stop=False)
                    for h in range(H):
                        half = (h % 2) * QK
                        cb = h * QK
                        for xt in range(XT):
                            nc.tensor.matmul(
                                pl[half:half + QK, cb:cb + QK],
                                khq[:, wi, xt, cb:cb + QK],
                                qsT[:, wi, xt, cb:cb + QK],
                                start=False, stop=(h == H - 1 and xt == XT - 1),
                                tile_position=(0, half))
                    e_t = mid.tile([128, C], bf16, tag="e")
                    nc.scalar.activation(e_t[:], pl[:], AF.Exp)
                    pls.append(pl); ets.append(e_t)

                attn = mid.tile([128, KT, 2, X], bf16, tag="attn")
                for wi in range(2):
                    e_t = ets[wi]
                    for t in range(KT):          # head pairs (2t, 2t+1)
                        pu = psU.tile([128, X + 2], f32, tag="pu")
                        for j in range(2):       # j=0 even head, j=1 odd head
                            h = 2 * t + j
                            half = j * QK
                            nc.tensor.matmul(
                                pu[half:half + QK, :],
                                e_t[half:half + QK, h * QK:(h + 1) * QK],
                                vplus[half:half + QK, t, wi, :],
                                start=True, stop=True,
                                tile_position=(half, half))
                        recip = small.tile([128, 1], f32, tag="recip")
                        nc.vector.reciprocal(recip[:], pu[:, X:X + 1])
                        nc.vector.tensor_scalar_mul(
                            attn[:, t, wi, :], pu[:, 0:X], recip[:])
                return attn

            def o_proj(attn, w0):
                ob = mid.tile([128, KT, 2, X], bf16, tag="ob")
                for ot in range(KT):
                    po = psVL.tile([128, 2, X], f32, tag="vl")
                    for kt in range(KT):
                        nc.tensor.matmul(
                            po[:],
                            wo_sb[:, kt, ot * 128:(ot + 1) * 128],
                            attn[:, kt, :, :],
                            start=(kt == 0), stop=(kt == KT - 1))
                    nc.scalar.activation(ob[:, ot], po[:], AF.Copy)
                nc.scalar.dma_start(
                    out[:, w0:w0 + 2, :].rearrange("(ot p) w x -> p ot (w x)",
                                                   p=128),
                    ob[:])

            cur_in = (qk0, v0)
            for ph in range(PAIRS + 1):
                if ph < PAIRS:
                    if ph + 1 < PAIRS:
                        next_in = load_inputs(ph + 1)
                    qk_t, v_t = cur_in
                    cur["qkt"] = qk_proj(qk_t)
                if ph > 0:
                    at = attention(*prev["qkt"], prev["vplus"])
                if ph < PAIRS:
                    cur["vplus"] = v_proj(v_t)
                    cur_in = next_in if ph + 1 < PAIRS else None
                if ph > 0:
                    o_proj(at, (ph - 1) * 2)
                prev, cur = cur, {}

    nc.compile()
    return nc


def _get_program():
    if "nc" not in _CACHE:
        _CACHE["nc"] = _build_program()
    return _CACHE["nc"]


def _make_in_maps(query, key_, value, Wq, Wk, Wv, Wo, q_emb, k_emb, v_emb):
    import ml_dtypes
    bf16 = ml_dtypes.bfloat16
    fp8 = ml_dtypes.float8_e4m3

    # fp8 weights: pre-scaled (2x / 8x) to clear the e4m3 subnormal floor;
    # compensated by the PSUM-evacuation scales inside the kernel.
    # layout [p, chunk, slot, o] with channel = chunk*256 + slot*128 + p.
    def w8(Wm, s):
        return np.ascontiguousarray(
            (Wm.T * s).reshape(2, 2, 128, C).transpose(2, 0, 1, 3).astype(fp8))

    wq8 = w8(Wq, 2.0)
    wk8 = w8(Wk, 8.0)
    wvt = np.ascontiguousarray(Wv.T.astype(bf16))
    wot = np.ascontiguousarray(Wo.T.astype(bf16))
    qe8 = np.ascontiguousarray(np.tile(q_emb, (1, H)).astype(bf16))
    ke2 = np.ascontiguousarray(np.concatenate([k_emb, k_emb], axis=1).astype(bf16))
    vet = np.ascontiguousarray(v_emb.T)

    def shardv(a, ws):
        # (C, X, WC) -> [pair, i, w, x] contiguous bf16
        return np.ascontiguousarray(
            a[:, :, ws].reshape(C, X, PAIRS, 2).transpose(2, 0, 3, 1).astype(bf16))

    in_maps = []
    for c in range(N_CORES):
        ws = slice(c * WC, (c + 1) * WC)
        # (C, X, WC) x2 -> [pair, p, qk, chunk, slot, wi, x] contiguous fp8
        qk = np.stack([query[:, :, ws], key_[:, :, ws]])  # (2, C, X, WC)
        qk8 = np.ascontiguousarray(
            qk.reshape(2, 2, 2, 128, X, PAIRS, 2)
            .transpose(5, 3, 0, 1, 2, 6, 4).astype(fp8))
        in_maps.append({
            "qk8in": qk8,
            "vin": shardv(value, ws),
            "wq8": wq8, "wk8": wk8, "wvt": wvt, "wot": wot,
            "qe8": qe8, "ke2": ke2, "vet": vet,
            "oned": np.ones((128, 4), bf16),
        })
    return in_maps


def _run(in_maps, trace=False):
    from concourse.bass_utils import run_bass_kernel_spmd
    nc = _get_program()
    return run_bass_kernel_spmd(nc, in_maps, list(range(N_CORES)), trace=trace)


def kernel(query, key_, value, Wq, Wk, Wv, Wo, q_emb, k_emb, v_emb):
    args = (query, key_, value, Wq, Wk, Wv, Wo, q_emb, k_emb, v_emb)
    in_maps = _make_in_maps(*[np.ascontiguousarray(a, np.float32) for a in args])
    res = _run(in_maps, trace=False)
    out = np.empty((C, X, W), np.float32)
    for c in range(N_CORES):
        out[:, :, c * WC:(c + 1) * WC] = (
            res.results[c]["out"].astype(np.float32).transpose(0, 2, 1))
    return out
